# revision 61
# baseline (speedup 1.0000x reference)
"""AttentionBlock3D kernel for 8 Trainium2 NeuronCores.

Problem (hardcoded): x [2, 256, 16, 16, 16] fp32, GroupNorm(8 groups) ->
qkv 1x1 conv -> 8-head attention over S=4096 -> proj -> residual.

Sharding: sequence-parallel. Core i handles batch b = i//4 and the
s-chunk [1024*(i%4), 1024*(i%4+1)) of the flattened spatial dim. Every
core recomputes GroupNorm stats and full k/v for its batch; q /
attention rows / proj / output are computed only for the core's own
s-chunk, so the host-side unshard is a pure concatenation.

Design notes (measured ~408us vs 738us for the original version):
  - The two s-halves of each head run as one merged 22-slot stream:
    scores+exp flow at the ACT cadence while a FIFO drains attn@v
    accumulation groups ~1 slot behind, so the U tail of half 0
    overlaps half 1's exp stream and ACT never starves at the half
    boundary. k/q replicas for head h+1 prefetch mid-head; the 1/Z
    broadcast matmuls of head h are deferred past head h+1's first
    scores group.
  - No K=1 bias matmuls: the k bias cancels exactly in softmax (it adds
    a per-s-column constant to scores); the v bias is folded into the
    output bias as proj_w @ qkv_b_v + proj_b; the q bias contributes
    scores += (k^T bq)[t], a per-t term folded multiplicatively into
    vt rows as f_t = exp(scale * (k^T bq)_t) (including the ones
    column, so Z stays consistent). All exact identities.
  - exp runs on ACT reading 3-bank PSUM score groups and writing fp16
    directly to SBUF (no separate DVE cast pass); both exp paths
    compute exp(x)/8 (softmax-shift-invariant constant shift).
  - 2 of 11 exp groups per (head, half) run on DVE instead via a 4-pass
    exp2 bit-trick (custom op computes ps + poly(frac), then add bias,
    AND-mask the f32 mantissa, truncating int32->int16 copy yields fp16
    bits). Their attn@v accumulation steps are reordered to the end of
    the (strictly ordered) U chain so PE never stalls on the slower
    DVE path.
  - attn @ v: per (head, half) 32 fp16 matmuls [K=128t, M=64, N=512s]
    accumulate over t-blocks; vt col 32 carries f_t so row 32 of U is
    the softmax denominator Z.
  - 1/Z via full-tile reciprocal_approx_fast (custom DVE ops silently
    no-op on partial-partition slices -- full [128, 512] tiles only),
    broadcast across partitions with two K=1 fp16 ones-matmuls.
"""

import numpy as np

B, C, S = 2, 256, 4096
NH, HD, G = 8, 32, 8
EPS = 1e-5
SC = 1024          # s-chunk length per core
NCORES = 8
NTB = S // 128     # 32 t-blocks
SCALE = HD ** -0.5
GN_N = (C // G) * S  # elements per group norm group

_cache = {}

PRESCALE = float(1024 * np.log2(np.e) * SCALE)
ACT_EXP_SCALE = float(np.log(2.0) / 1024.0)
# Both exp paths compute exp(x)/4 (shift of 2048 in ps units = 2 octaves);
# softmax-shift-invariant, keeps headroom for the DVE bit-trick range.
PS_SHIFT = 2048.0
EXP_SHIFT = float(-PS_SHIFT * np.log(2.0) / 1024.0)  # ACT bias: -ln(4)
EXPQ1 = -0.01106242
EXPQ2 = 0.0003354418
EXPK2 = 8403879.0
M2F = 1.5 * 2 ** 33
EXP_CORE = None


def _register_exp_core():
    global EXP_CORE
    if EXP_CORE is not None:
        return
    import re
    from concourse import dve_ops
    from concourse.dve_spec import Spec, Src0, C0, C1, C2, C3
    from concourse.dve_ops import DveOp, _spill_c3_to_src1

    def _ref_exp_core(in0, in1, c0, c1, c2):
        T2 = (in0.astype(np.float32) - np.float32(c2)).astype(np.float32)
        u0 = (T2 + np.float32(M2F)).astype(np.float32)
        wmk = (u0 - np.float32(M2F)).astype(np.float32)
        fk = (T2 - wmk).astype(np.float32)
        return fk * (np.float32(c0) * fk + np.float32(c1))

    def _ref_exp_core2(in0, in1, c0, c1, c2):
        return (in0.astype(np.float32)
                + _ref_exp_core(in0, in1, c0, c1, c2)).astype(np.float32)

    _T2 = Src0 - C2
    _u0 = _T2 + C3
    _wmk = _u0 - C3
    _fk = _T2 - _wmk
    # scr = ps + poly(fk): releases the PSUM operand after this one pass
    _body = Src0 + _fk * (C0 * _fk + C1)
    op = DveOp("EXP_CORE2_ANT",
               Spec(body=_spill_c3_to_src1(_body), reference=_ref_exp_core2),
               subdim=False, uops_sha={})
    dve_ops.OPS.append(op)
    dve_ops._SUB_OPCODE_FOR_NAME[op.name] = (
        dve_ops._CUSTOM_DVE_ROW_BASE + len(dve_ops.OPS) - 1)
    dve_ops.CUSTOM_DVE_SPECS[op.name] = op.spec
    for ver in ("v3", "v4"):
        try:
            op.compile(ver)
        except ValueError as e:
            m = re.search(r'uops_sha\["' + ver + r'"\]="([0-9a-f]+)"', str(e))
            op.uops_sha[ver] = m.group(1)
    EXP_CORE = op


def _build_nc(debug=False):
    import concourse.bass as bass
    import concourse.bacc as bacc
    import concourse.tile as tile
    from concourse import mybir
    from concourse.masks import make_identity

    f32 = mybir.dt.float32
    f16 = mybir.dt.float16
    f8 = mybir.dt.float8e4
    i32 = mybir.dt.int32
    i16 = mybir.dt.int16
    AF = mybir.ActivationFunctionType
    ALU = mybir.AluOpType
    AX = mybir.AxisListType
    DR = mybir.MatmulPerfMode.DoubleRow

    _register_exp_core()
    nc = bacc.Bacc()
    dx = nc.declare_dram_parameter("x_full", [C, S], f32, isOutput=False)
    dxq = nc.declare_dram_parameter("xq", [C, SC], f32, isOutput=False)
    dgw = nc.declare_dram_parameter("gn_w", [C], f32, isOutput=False)
    dgb = nc.declare_dram_parameter("gn_b", [C], f32, isOutput=False)
    dqkvw = nc.declare_dram_parameter("qkv_w", [3 * C, C], f32, isOutput=False)
    dqkvb = nc.declare_dram_parameter("qkv_b", [3 * C], f32, isOutput=False)
    dpw = nc.declare_dram_parameter("proj_w", [C, C], f32, isOutput=False)
    dpb = nc.declare_dram_parameter("proj_b", [C], f32, isOutput=False)
    dout = nc.declare_dram_parameter("out", [C, SC], f32, isOutput=True)
    if debug:
        dbg = {}
        for nm, shp, dt_ in [("dbg_pb2", [128, 2], f32), ("dbg_fT", [128, NTB], f32),
                             ("dbg_vt", [128, 512], f16),
                             ("dbg_exp", [128, 512], f16), ("dbg_usb", [128, 512], f32),
                             ("dbg_zsb", [128, 512], f32), ("dbg_zrh", [128, 512], f16),
                             ("dbg_stage", [128, 512], f16), ("dbg_ao", [128, SC], f16),
                             ("dbg_q", [128, 512], f16), ("dbg_k", [128, 512], f16)]:
            dbg[nm] = nc.declare_dram_parameter(nm, shp, dt_, isOutput=True)

    from contextlib import ExitStack
    with tile.TileContext(nc) as tc, ExitStack() as ctx:
        singles = ctx.enter_context(tc.tile_pool(name="singles", bufs=1))
        # PSUM pools: 2x3 + 1 + 1 = 8 banks exactly.
        ps_pool = ctx.enter_context(tc.tile_pool(name="ps", bufs=2, space="PSUM"))
        u_pool = ctx.enter_context(tc.tile_pool(name="u", bufs=1, space="PSUM"))
        zr_pool = ctx.enter_context(tc.tile_pool(name="zr", bufs=1, space="PSUM"))
        kq = ctx.enter_context(tc.tile_pool(name="kq", bufs=2))
        vt_pool = ctx.enter_context(tc.tile_pool(name="vtp", bufs=1))
        # preamble-scoped pools (released before expS allocates)
        pre = ExitStack()
        xp = pre.enter_context(tc.tile_pool(name="xp", bufs=2))
        hp = pre.enter_context(tc.tile_pool(name="hp", bufs=2))
        wstage = pre.enter_context(tc.tile_pool(name="wstage", bufs=2))

        ones = singles.tile([128, 512], f32, tag="ones", name="ones")
        nc.vector.memset(ones, 1.0)
        zero_b = singles.tile([128, 1], f32, tag="zb", name="zb")
        nc.vector.memset(zero_b, 0.0)
        expb = singles.tile([128, 1], f32, tag="expb", name="expb")
        nc.vector.memset(expb, EXP_SHIFT)
        ones16 = singles.tile([128, 128], f16, tag="ones16", name="ones16")
        nc.vector.memset(ones16, 1.0)
        ident = singles.tile([128, 128], f32, tag="ident", name="ident")
        make_identity(nc, ident)

        # ---- small parameter loads ----
        gw = [singles.tile([128, 1], f32, tag=f"gw{i}", name=f"gw{i}") for i in range(2)]
        gb = [singles.tile([128, 1], f32, tag=f"gb{i}", name=f"gb{i}") for i in range(2)]
        pb = [singles.tile([128, 1], f32, tag=f"pb{i}", name=f"pb{i}") for i in range(2)]
        bqT = [singles.tile([128, 1], f32, tag=f"bqT{i}", name=f"bqT{i}") for i in range(2)]
        bvT = [singles.tile([128, 1], f32, tag=f"bvT{i}", name=f"bvT{i}") for i in range(2)]
        for ct in range(2):
            nc.sync.dma_start(out=gw[ct], in_=dgw[128 * ct:128 * (ct + 1)])
            nc.sync.dma_start(out=gb[ct], in_=dgb[128 * ct:128 * (ct + 1)])
            nc.sync.dma_start(out=pb[ct], in_=dpb[128 * ct:128 * (ct + 1)])
            nc.sync.dma_start(out=bqT[ct], in_=dqkvb[128 * ct:128 * (ct + 1)])
            nc.sync.dma_start(out=bvT[ct], in_=dqkvb[2 * C + 128 * ct:2 * C + 128 * (ct + 1)])

        # ---- weight transposes (PE) ----
        # wqkvT[ct][c, o] = qkv_w[o, 128*ct + c]; fp16 tiles [128, 768]
        wqkvT = [singles.tile([128, 3 * C], f16, tag=f"wqkvT{i}", name=f"wqkvT{i}") for i in range(2)]
        wpT = [singles.tile([128, C], f16, tag=f"wpT{i}", name=f"wpT{i}") for i in range(2)]
        for rt in range(6):  # qkv_w row-tiles [128, 256]
            wle = wstage.tile([128, C], f32, tag="wle", name="wle")
            nc.sync.dma_start(out=wle, in_=dqkvw[128 * rt:128 * (rt + 1), :])
            for ct in range(2):
                pt = ps_pool.tile([128, 1536], f32, tag="ps", name="ps")
                nc.tensor.transpose(pt[:, 0:128], wle[:, 128 * ct:128 * (ct + 1)], ident)
                nc.vector.tensor_copy(
                    out=wqkvT[ct][:, 128 * rt:128 * (rt + 1)], in_=pt[:, 0:128])
        for rt in range(2):
            wle = wstage.tile([128, C], f32, tag="wle", name="wle")
            nc.sync.dma_start(out=wle, in_=dpw[128 * rt:128 * (rt + 1), :])
            for ct in range(2):
                pt = ps_pool.tile([128, 1536], f32, tag="ps", name="ps")
                nc.tensor.transpose(pt[:, 0:128], wle[:, 128 * ct:128 * (ct + 1)], ident)
                nc.vector.tensor_copy(
                    out=wpT[ct][:, 128 * rt:128 * (rt + 1)], in_=pt[:, 0:128])

        # pb2 = proj_b + proj_w @ bv  (exact fold of the v bias)
        bqT16 = [singles.tile([128, 1], f16, tag=f"bqT16_{i}", name=f"bqT16_{i}") for i in range(2)]
        bvT16 = [singles.tile([128, 1], f16, tag=f"bvT16_{i}", name=f"bvT16_{i}") for i in range(2)]
        for ct in range(2):
            nc.vector.tensor_copy(out=bqT16[ct], in_=bqT[ct])
            nc.vector.tensor_copy(out=bvT16[ct], in_=bvT[ct])
        pb2 = [singles.tile([128, 1], f32, tag=f"pb2_{i}", name=f"pb2_{i}") for i in range(2)]
        for ct in range(2):
            bvp = zr_pool.tile([128, 512], f32, tag="zr", name="bvp")
            for kc in range(2):
                nc.tensor.matmul(bvp[:, 0:1], wpT[kc][:, 128 * ct:128 * (ct + 1)],
                                 bvT16[kc], start=(kc == 0), stop=(kc == 1))
            nc.vector.tensor_add(out=pb2[ct], in0=pb[ct], in1=bvp[:, 0:1])
        if debug:
            pbs = wstage.tile([128, 2], f32, tag="pbs", name="pbs")
            nc.vector.tensor_copy(out=pbs[:, 0:1], in_=pb2[0])
            nc.vector.tensor_copy(out=pbs[:, 1:2], in_=pb2[1])
            nc.sync.dma_start(out=dbg["dbg_pb2"][:, :], in_=pbs)

        # ---- GroupNorm stats ----
        x_sb = [xp.tile([128, S], f32, tag="x", name="x") for _ in range(2)]
        h_sb = [hp.tile([128, S], f16, tag="h", name="h") for _ in range(2)]
        stats = [wstage.tile([128, 2], f32, tag=f"st{i}", name=f"st{i}") for i in range(2)]
        for ct in range(2):
            nc.sync.dma_start(out=x_sb[ct], in_=dx[128 * ct:128 * (ct + 1), :])
            nc.vector.tensor_reduce(
                out=stats[ct][:, 0:1], in_=x_sb[ct], axis=AX.X, op=ALU.add)
            # sum(x^2) in one ACT pass (h_sb is scratch, overwritten later)
            nc.scalar.activation(out=h_sb[ct], in_=x_sb[ct], func=AF.Square,
                                 accum_out=stats[ct][:, 1:2])
        st_ps = zr_pool.tile([1, 512], f32, tag="zr", name="zr")
        for ct in range(2):
            nc.tensor.transpose(st_ps[0:1, 128 * ct:128 * (ct + 1)],
                                stats[ct][:, 0:1], ident)
            nc.tensor.transpose(st_ps[0:1, 256 + 128 * ct:256 + 128 * (ct + 1)],
                                stats[ct][:, 1:2], ident)
        gstats = singles.tile([1, 16], f32, tag="gstats", name="gstats")
        nc.vector.tensor_reduce(
            out=gstats,
            in_=st_ps.rearrange("p (k g c) -> p k g c", k=2, g=G),
            axis=AX.X, op=ALU.add)
        mu = singles.tile([1, G], f32, tag="mu", name="mu")
        varv = singles.tile([1, G], f32, tag="varv", name="varv")
        rstd = singles.tile([1, G], f32, tag="rstd", name="rstd")
        mrs = singles.tile([1, G], f32, tag="mrs", name="mrs")
        eps_sb2 = singles.tile([128, 1], f32, tag="eps2", name="eps2")
        nc.vector.memset(eps_sb2, EPS)
        nc.vector.tensor_scalar_mul(out=mu, in0=gstats[:, 0:G], scalar1=1.0 / GN_N)
        nc.vector.tensor_scalar_mul(out=varv, in0=gstats[:, G:2 * G], scalar1=1.0 / GN_N)
        musq = singles.tile([1, G], f32, tag="musq", name="musq")
        nc.vector.tensor_mul(out=musq, in0=mu, in1=mu)
        nc.vector.tensor_sub(out=varv, in0=varv, in1=musq)
        vb_ps = zr_pool.tile([128, 512], f32, tag="zr", name="vb_ps")
        nc.tensor.matmul(vb_ps[:, 0:G], ones[0:1, 0:128], varv,
                         start=True, stop=True)
        sdb = singles.tile([128, G], f32, tag="sdb", name="sdb")
        nc.scalar.activation(out=sdb, in_=vb_ps[:, 0:G], func=AF.Sqrt,
                             bias=eps_sb2)
        nc.vector.reciprocal(out=rstd, in_=sdb[0:1, :])
        nc.vector.tensor_mul(out=mrs, in0=mu, in1=rstd)

        # broadcast rstd/mrs to per-channel A, Bb
        A = [singles.tile([128, 1], f32, tag=f"A{i}", name=f"A{i}") for i in range(2)]
        Bb = [singles.tile([128, 1], f32, tag=f"B{i}", name=f"B{i}") for i in range(2)]
        for ct in range(2):
            arep = zr_pool.tile([128, 2], f32, tag="zr", name="zr")
            for g4 in range(4):
                g = 4 * ct + g4
                nc.tensor.matmul(
                    arep[32 * g4:32 * (g4 + 1), 0:1], ones[0:1, 0:32],
                    rstd[:, g:g + 1], start=True, stop=True,
                    tile_position=(0, 32 * g4))
                nc.tensor.matmul(
                    arep[32 * g4:32 * (g4 + 1), 1:2], ones[0:1, 0:32],
                    mrs[:, g:g + 1], start=True, stop=True,
                    tile_position=(0, 32 * g4))
            nc.vector.tensor_mul(out=A[ct], in0=arep[:, 0:1], in1=gw[ct])
            tmp = wstage.tile([128, 1], f32, tag="tmpB", name="tmpB")
            nc.vector.tensor_mul(out=tmp, in0=arep[:, 1:2], in1=gw[ct])
            nc.vector.tensor_sub(out=Bb[ct], in0=gb[ct], in1=tmp)

        # ---- apply GN ----
        xq_sb = [xp.tile([128, SC], f32, tag="xq", name="xq") for _ in range(2)]
        hq_sb = [hp.tile([128, SC], f16, tag="hq", name="hq") for _ in range(2)]
        xpb = [singles.tile([128, SC], f32, tag=f"xpb{i}", name=f"xpb{i}") for i in range(2)]
        for ct in range(2):
            nc.sync.dma_start(out=xq_sb[ct], in_=dxq[128 * ct:128 * (ct + 1), :])
            nc.scalar.activation(out=h_sb[ct], in_=x_sb[ct], func=AF.Identity,
                                 bias=Bb[ct], scale=A[ct])
            nc.scalar.activation(out=hq_sb[ct], in_=xq_sb[ct], func=AF.Identity,
                                 bias=Bb[ct], scale=A[ct])
            nc.scalar.activation(out=xpb[ct], in_=xq_sb[ct], func=AF.Identity,
                                 bias=pb2[ct])

        # ---- q, k matmuls (fp16, no bias) ----
        q_sb = [kq.tile([128, SC], f16, tag="q", name="q") for _ in range(2)]
        k_sb = [kq.tile([128, S], f16, tag="k", name="k") for _ in range(2)]
        for ct in range(2):
            pq = ps_pool.tile([128, 1536], f32, tag="ps", name="ps")
            for n in range(2):
                for kc in range(2):
                    nc.tensor.matmul(
                        pq[:, 512 * n:512 * (n + 1)],
                        wqkvT[kc][:, 128 * ct:128 * (ct + 1)],
                        hq_sb[kc][:, 512 * n:512 * (n + 1)],
                        start=(kc == 0), stop=(kc == 1))
            nc.vector.tensor_scalar(out=q_sb[ct], in0=pq[:, 0:SC],
                                    scalar1=PRESCALE, scalar2=None, op0=ALU.mult)
        for ct in range(2):
            for chunk in range(4):  # 4 chunks of 1024 cols
                pk = ps_pool.tile([128, 1536], f32, tag="ps", name="ps")
                for n in range(2):
                    cl = 1024 * chunk + 512 * n
                    for kc in range(2):
                        nc.tensor.matmul(
                            pk[:, 512 * n:512 * (n + 1)],
                            wqkvT[kc][:, C + 128 * ct:C + 128 * (ct + 1)],
                            h_sb[kc][:, cl:cl + 512],
                            start=(kc == 0), stop=(kc == 1))
                nc.vector.tensor_copy(
                    out=k_sb[ct][:, 1024 * chunk:1024 * (chunk + 1)],
                    in_=pk[:, 0:1024])

        # ---- f_t = exp(SCALE * (k^T bq)_t): q-bias fold, t in partitions ----
        fps = zr_pool.tile([128, 512], f32, tag="zr", name="fps")
        for tb in range(NTB):
            for kc in range(2):
                nc.tensor.matmul(
                    fps[:, tb:tb + 1],
                    k_sb[kc][:, 128 * tb:128 * (tb + 1)], bqT16[kc],
                    start=(kc == 0), stop=(kc == 1))
        fT = singles.tile([128, NTB], f32, tag="fT", name="fT")
        nc.scalar.activation(out=fT, in_=fps[:, 0:NTB], func=AF.Exp,
                             scale=SCALE, bias=zero_b)
        if debug:
            nc.sync.dma_start(out=dbg["dbg_fT"][:, :], in_=fT)

        # ---- vt: [t(128p), tb, h, 64] fp16; col 0:32 = f*v, col 32 = f ----
        vt = vt_pool.tile([128, NTB, NH, 64], f16, tag="vt", name="vt")
        for tb in range(NTB):
            pv = (u_pool.tile([128, 512], f32, tag="u", name="pv")
                  if tb % 2 == 0 else
                  zr_pool.tile([128, 512], f32, tag="zr", name="pv"))
            for kc in range(2):
                nc.tensor.matmul(
                    pv[:, 0:256],
                    h_sb[kc][:, 128 * tb:128 * (tb + 1)],
                    wqkvT[kc][:, 2 * C:3 * C],
                    start=(kc == 0), stop=(kc == 1))
            nc.scalar.activation(
                out=vt[:, tb, :, 0:32],
                in_=pv[:, 0:256].rearrange("p (h d) -> p h d", h=NH),
                func=AF.Identity, bias=zero_b, scale=fT[:, tb:tb + 1])
        for h in range(NH):
            nc.gpsimd.tensor_copy(out=vt[:, :, h, 32:33].rearrange("p t o -> p (t o)"),
                                  in_=fT)
        if debug:
            nc.sync.dma_start(out=dbg["dbg_vt"][:, :],
                              in_=vt[:, 0, :, :].rearrange("p h d -> p (h d)"))
            nc.sync.dma_start(out=dbg["dbg_q"][:, :], in_=q_sb[0][:, 0:512])
            nc.sync.dma_start(out=dbg["dbg_k"][:, :], in_=k_sb[0][:, 0:512])

        # ---- attention ----
        pre.close()  # release x/h/staging SBUF for expS
        exps_pool = ctx.enter_context(tc.tile_pool(name="exps", bufs=1))
        rep = ctx.enter_context(tc.tile_pool(name="rep", bufs=2))
        zt_pool = ctx.enter_context(tc.tile_pool(name="zt", bufs=2))
        stg_pool = ctx.enter_context(tc.tile_pool(name="stg", bufs=2))
        scr_pool = ctx.enter_context(tc.tile_pool(name="scr", bufs=1))
        osb_pool = ctx.enter_context(tc.tile_pool(name="osb", bufs=2))
        expS = exps_pool.tile([128, 2, NTB, 512], f16, tag="expS", name="expS")
        m2t = exps_pool.tile([128, 1], f32, tag="m2t", name="m2t")
        nc.vector.memset(m2t, M2F)
        attnout = [kq.tile([128, SC], f16, tag="ao", name="ao") for _ in range(2)]
        # t-block groups of 3 (last group 2) per s-half
        groups = [list(range(i, min(i + 3, NTB))) for i in range(0, NTB, 3)]
        # Per-half DVE exp groups. Spacing between consecutive DVE chains
        # must exceed the ~4.4us chain latency (3+ slots at the 1.45us ACT
        # cadence) or the DVE queue backs up and queued custom ops hold
        # their PSUM score banks, stalling PE.
        DVE_GROUPS = {0: (3, 8), 1: (3, 7)}

        def emit_u(h, half, grp, first, last):
            for i, tb in enumerate(grp):
                nc.tensor.matmul(
                    U[64 * half:64 * half + 33, :],
                    vt[:, tb, h, 0:33],
                    expS[:, half, tb, :],
                    start=(first and i == 0),
                    stop=(last and i == len(grp) - 1),
                    tile_position=(0, 64 * half))

        reps = {}

        def prefetch_rep(hh):
            pkt, pkr = hh // 4, 32 * (hh % 4)
            k3p = rep.tile([96, S], f16, tag="k3", name="k3")
            q3p = rep.tile([96, SC], f16, tag="q3", name="q3")
            for i in range(3):
                nc.gpsimd.dma_start(out=k3p[32 * i:32 * (i + 1), :],
                                    in_=k_sb[pkt][pkr:pkr + 32, :])
                nc.gpsimd.dma_start(out=q3p[32 * i:32 * (i + 1), :],
                                    in_=q_sb[pkt][pkr:pkr + 32, :])
            reps[hh] = (k3p, q3p)

        prefetch_rep(0)
        pending_z = None
        for h in range(NH):
            kt, kr = h // 4, 32 * (h % 4)
            k3, q3 = reps.pop(h)
            U = u_pool.tile([128, 512], f32, tag="u", name="u")
            # Single 22-slot stream over (half, group): the U-drain FIFO lags
            # behind the exp stream, so half 0's U tail drains while half 1's
            # scores/exps flow and ACT never starves at the half boundary.
            nemit = {0: 0, 1: 0}
            fifo = []          # (half, grp) ready for U emission, in order
            pend_dve = []      # (ready_slot, half, grp)
            units = [(hf, gi) for hf in range(2) for gi in range(len(groups))]

            def drain(half_grp, last):
                half, g = half_grp
                emit_u(h, half, g, nemit[half] == 0, last)
                nemit[half] += len(g)

            for slot, (half, gi) in enumerate(units):
                grp = groups[gi]
                ng = len(grp)
                ps = ps_pool.tile([128, 1536], f32, tag="ps", name="ps")
                for i, tb in enumerate(grp):
                    nc.tensor.matmul(
                        ps[:, 512 * i:512 * (i + 1)],
                        k3[32 * i:32 * (i + 1), 128 * tb:128 * (tb + 1)],
                        q3[32 * i:32 * (i + 1), 512 * half:512 * (half + 1)],
                        start=True, stop=True, tile_position=(32 * i, 0))
                while pend_dve and pend_dve[0][0] <= slot:
                    _, hf, g = pend_dve.pop(0)
                    fifo.append((hf, g))
                if fifo:
                    drain(fifo.pop(0), False)
                if half == 0 and gi == 0 and pending_z is not None:
                    pending_z()
                    pending_z = None
                if half == 1 and gi == 0 and h + 1 < NH:
                    prefetch_rep(h + 1)
                if gi in DVE_GROUPS[half]:
                    # 4-pass fp16 exp2 bit-trick on DVE; pass 1 frees ps
                    scr1 = scr_pool.tile([128, 1536], f32, tag="scr1", name="scr1")
                    scr2 = scr_pool.tile([128, 1536], f32, tag="scr2", name="scr2")
                    nc.vector._custom_dve(
                        EXP_CORE, out=scr1[:, 0:512 * ng],
                        in0=ps[:, 0:512 * ng], in1=m2t,
                        s0=EXPQ2, s1=EXPQ1, imm2=512.0 + PS_SHIFT)
                    nc.vector.tensor_scalar(
                        out=scr2[:, 0:512 * ng], in0=scr1[:, 0:512 * ng],
                        scalar1=EXPK2 - PS_SHIFT, scalar2=None, op0=ALU.add)
                    nc.vector.tensor_scalar(
                        out=scr1.bitcast(i32)[:, 0:512 * ng],
                        in0=scr2.bitcast(i32)[:, 0:512 * ng],
                        scalar1=0x7FFFFF, scalar2=None,
                        op0=ALU.bitwise_and)
                    nc.vector.tensor_copy(
                        out=expS.bitcast(i16)[:, half, grp[0]:grp[0] + ng, :],
                        in_=scr1.bitcast(i32)[:, 0:512 * ng].rearrange(
                            "p (t s) -> p t s", s=512))
                    pend_dve.append((slot + 4, half, grp))
                else:
                    nc.scalar.activation(
                        out=expS[:, half, grp[0]:grp[0] + ng, :],
                        in_=ps[:, 0:512 * ng].rearrange("p (t s) -> p t s", s=512),
                        func=AF.Exp, scale=ACT_EXP_SCALE, bias=expb)
                    fifo.append((half, grp))
            tail = fifo + [(hf, g) for _, hf, g in pend_dve]
            left = {0: 0, 1: 0}
            for hf, g in tail:
                left[hf] += 1
            for hf, g in tail:
                left[hf] -= 1
                drain((hf, g), left[hf] == 0)
            # U -> SBUF; batched 1/Z on full tile (rows 32 & 96 are Z).
            # Only the U reader (u_sb copy) + DVE/Pool work emit now; the
            # PE broadcast matmuls are deferred past the next head's first
            # scores group so PE never waits on the recip chain.
            u_sb = zt_pool.tile([128, 512], f32, tag="usb", name="usb")
            nc.vector.tensor_copy(out=u_sb[0:33, :], in_=U[0:33, :])
            nc.vector.tensor_copy(out=u_sb[64:97, :], in_=U[64:97, :])
            zrc = zt_pool.tile([128, 512], f32, tag="zrc", name="zrc")
            zrh = zt_pool.tile([128, 512], f16, tag="zrh", name="zrh")
            nc.vector.reciprocal_approx_fast(out=zrc, in_=u_sb)
            nc.gpsimd.tensor_copy(out=zrh, in_=zrc)

            def z_tail(h=h, kt=kt, kr=kr, u_sb=u_sb, zrc=zrc, zrh=zrh):
                zrep = zr_pool.tile([128, 512], f32, tag="zr", name="zrep")
                nc.tensor.matmul(zrep[0:64, :], ones16[32:33, 0:64],
                                 zrh[32:33, :], start=True, stop=True,
                                 tile_position=(32, 0))
                nc.tensor.matmul(zrep[64:128, :], ones16[96:97, 64:128],
                                 zrh[96:97, :], start=True, stop=True,
                                 tile_position=(96, 64))
                stage = stg_pool.tile([128, 512], f16, tag="stage", name="stage")
                nc.vector.tensor_mul(out=stage, in0=u_sb, in1=zrep)
                if debug and h == 0:
                    nc.sync.dma_start(out=dbg["dbg_usb"][:, :], in_=u_sb)
                    nc.sync.dma_start(out=dbg["dbg_zsb"][:, :], in_=zrc)
                    nc.sync.dma_start(out=dbg["dbg_zrh"][:, :], in_=zrh)
                    nc.sync.dma_start(out=dbg["dbg_stage"][:, :], in_=stage)
                nc.gpsimd.dma_start(out=attnout[kt][kr:kr + 32, 0:512],
                                    in_=stage[0:32, :])
                nc.gpsimd.dma_start(out=attnout[kt][kr:kr + 32, 512:1024],
                                    in_=stage[64:96, :])

            pending_z = z_tail
        if pending_z is not None:
            pending_z()
            pending_z = None

        if debug:
            nc.sync.dma_start(out=dbg["dbg_ao"][:, :], in_=attnout[0])

        # ---- proj + residual ----
        out_sb = [osb_pool.tile([128, SC], f32, tag="osb", name="osb") for _ in range(2)]
        for ct in range(2):
            pp = ps_pool.tile([128, 1536], f32, tag="ps", name="ps")
            for n in range(2):
                for kc in range(2):
                    nc.tensor.matmul(
                        pp[:, 512 * n:512 * (n + 1)],
                        wpT[kc][:, 128 * ct:128 * (ct + 1)],
                        attnout[kc][:, 512 * n:512 * (n + 1)],
                        start=(kc == 0), stop=(kc == 1))
            nc.vector.tensor_add(out=out_sb[ct], in0=pp[:, 0:SC], in1=xpb[ct])
            nc.sync.dma_start(out=dout[128 * ct:128 * (ct + 1), :],
                              in_=out_sb[ct])

    nc.finalize()
    return nc


def make_in_maps(inputs):
    xf = np.ascontiguousarray(
        np.asarray(inputs["x"], dtype=np.float32)).reshape(B, C, S)
    in_maps = []
    for i in range(NCORES):
        b, sc = i // 4, SC * (i % 4)
        in_maps.append({
            "x_full": xf[b],
            "xq": np.ascontiguousarray(xf[b][:, sc:sc + SC]),
            "gn_w": np.asarray(inputs["gn_w"], np.float32),
            "gn_b": np.asarray(inputs["gn_b"], np.float32),
            "qkv_w": np.asarray(inputs["qkv_w"], np.float32),
            "qkv_b": np.asarray(inputs["qkv_b"], np.float32),
            "proj_w": np.asarray(inputs["proj_w"], np.float32),
            "proj_b": np.asarray(inputs["proj_b"], np.float32),
        })
    return in_maps


def kernel(x, gn_w, gn_b, qkv_w, qkv_b, proj_w, proj_b):
    from concourse.bass_utils import run_bass_kernel_spmd

    if "nc" not in _cache:
        _cache["nc"] = _build_nc()
    nc = _cache["nc"]

    in_maps = make_in_maps(dict(x=x, gn_w=gn_w, gn_b=gn_b, qkv_w=qkv_w,
                                qkv_b=qkv_b, proj_w=proj_w, proj_b=proj_b))
    res = run_bass_kernel_spmd(nc, in_maps, list(range(NCORES))).results
    out = np.empty((B, C, S), np.float32)
    for i in range(NCORES):
        b, sc = i // 4, SC * (i % 4)
        out[b][:, sc:sc + SC] = res[i]["out"]
    return out.reshape(B, C, 16, 16, 16)


# revision 62
# speedup vs baseline: 1.2383x; 1.2383x over previous
"""AttentionBlock3D kernel for 8 Trainium2 NeuronCores.

Problem (hardcoded): x [2, 256, 16, 16, 16] fp32, GroupNorm(8 groups) ->
qkv 1x1 conv -> 8-head attention over S=4096 -> proj -> residual.

Sharding: sequence-parallel. Core i handles batch b = i//4 and the
s-chunk [1024*(i%4), 1024*(i%4+1)) of the flattened spatial dim. Every
core recomputes GroupNorm stats and full k/v for its batch; q /
attention rows / proj / output are computed only for the core's own
s-chunk, so the host-side unshard is a pure concatenation.

Design notes (measured ~408us vs 738us for the original version):
  - The two s-halves of each head run as one merged 22-slot stream:
    scores+exp flow at the ACT cadence while a FIFO drains attn@v
    accumulation groups ~1 slot behind, so the U tail of half 0
    overlaps half 1's exp stream and ACT never starves at the half
    boundary. k/q replicas for head h+1 prefetch mid-head; the 1/Z
    broadcast matmuls of head h are deferred past head h+1's first
    scores group.
  - No K=1 bias matmuls: the k bias cancels exactly in softmax (it adds
    a per-s-column constant to scores); the v bias is folded into the
    output bias as proj_w @ qkv_b_v + proj_b; the q bias contributes
    scores += (k^T bq)[t], a per-t term folded multiplicatively into
    vt rows as f_t = exp(scale * (k^T bq)_t) (including the ones
    column, so Z stays consistent). All exact identities.
  - exp runs on ACT reading 3-bank PSUM score groups and writing fp16
    directly to SBUF (no separate DVE cast pass); both exp paths
    compute exp(x)/8 (softmax-shift-invariant constant shift).
  - 2 of 11 exp groups per (head, half) run on DVE instead via a 4-pass
    exp2 bit-trick (custom op computes ps + poly(frac), then add bias,
    AND-mask the f32 mantissa, truncating int32->int16 copy yields fp16
    bits). Their attn@v accumulation steps are reordered to the end of
    the (strictly ordered) U chain so PE never stalls on the slower
    DVE path.
  - attn @ v: per (head, half) 32 fp16 matmuls [K=128t, M=64, N=512s]
    accumulate over t-blocks; vt col 32 carries f_t so row 32 of U is
    the softmax denominator Z.
  - 1/Z via full-tile reciprocal_approx_fast (custom DVE ops silently
    no-op on partial-partition slices -- full [128, 512] tiles only),
    broadcast across partitions with two K=1 fp16 ones-matmuls.
"""

import numpy as np

B, C, S = 2, 256, 4096
NH, HD, G = 8, 32, 8
EPS = 1e-5
SC = 1024          # s-chunk length per core
NCORES = 8
NTB = S // 128     # 32 t-blocks
SCALE = HD ** -0.5
GN_N = (C // G) * S  # elements per group norm group

_cache = {}

PRESCALE = float(1024 * np.log2(np.e) * SCALE)
ACT_EXP_SCALE = float(np.log(2.0) / 1024.0)
# Both exp paths compute exp(x)/4 (shift of 2048 in ps units = 2 octaves);
# softmax-shift-invariant, keeps headroom for the DVE bit-trick range.
PS_SHIFT = 2048.0
EXP_SHIFT = float(-PS_SHIFT * np.log(2.0) / 1024.0)  # ACT bias: -ln(4)
EXPQ1 = -0.01106242
EXPQ2 = 0.0003354418
EXPK2 = 8403879.0
M2F = 1.5 * 2 ** 33
EXP_CORE = None


def _register_exp_core():
    global EXP_CORE
    if EXP_CORE is not None:
        return
    import re
    from concourse import dve_ops
    from concourse.dve_spec import Spec, Src0, C0, C1, C2, C3
    from concourse.dve_ops import DveOp, _spill_c3_to_src1

    def _ref_exp_core(in0, in1, c0, c1, c2):
        T2 = (in0.astype(np.float32) - np.float32(c2)).astype(np.float32)
        u0 = (T2 + np.float32(M2F)).astype(np.float32)
        wmk = (u0 - np.float32(M2F)).astype(np.float32)
        fk = (T2 - wmk).astype(np.float32)
        return fk * (np.float32(c0) * fk + np.float32(c1))

    def _ref_exp_core2(in0, in1, c0, c1, c2):
        return (in0.astype(np.float32)
                + _ref_exp_core(in0, in1, c0, c1, c2)).astype(np.float32)

    _T2 = Src0 - C2
    _u0 = _T2 + C3
    _wmk = _u0 - C3
    _fk = _T2 - _wmk
    # scr = ps + poly(fk): releases the PSUM operand after this one pass
    _body = Src0 + _fk * (C0 * _fk + C1)
    op = DveOp("EXP_CORE2_ANT",
               Spec(body=_spill_c3_to_src1(_body), reference=_ref_exp_core2),
               subdim=False, uops_sha={})
    dve_ops.OPS.append(op)
    dve_ops._SUB_OPCODE_FOR_NAME[op.name] = (
        dve_ops._CUSTOM_DVE_ROW_BASE + len(dve_ops.OPS) - 1)
    dve_ops.CUSTOM_DVE_SPECS[op.name] = op.spec
    for ver in ("v3", "v4"):
        try:
            op.compile(ver)
        except ValueError as e:
            m = re.search(r'uops_sha\["' + ver + r'"\]="([0-9a-f]+)"', str(e))
            op.uops_sha[ver] = m.group(1)
    EXP_CORE = op


def _build_nc(debug=False):
    import concourse.bass as bass
    import concourse.bacc as bacc
    import concourse.tile as tile
    from concourse import mybir
    from concourse.masks import make_identity

    f32 = mybir.dt.float32
    f16 = mybir.dt.float16
    f8 = mybir.dt.float8e4
    i32 = mybir.dt.int32
    i16 = mybir.dt.int16
    AF = mybir.ActivationFunctionType
    ALU = mybir.AluOpType
    AX = mybir.AxisListType
    DR = mybir.MatmulPerfMode.DoubleRow

    _register_exp_core()
    nc = bacc.Bacc()
    dx = nc.declare_dram_parameter("x_full", [C, S], f32, isOutput=False)
    dxq = nc.declare_dram_parameter("xq", [C, SC], f32, isOutput=False)
    dgw = nc.declare_dram_parameter("gn_w", [C], f32, isOutput=False)
    dgb = nc.declare_dram_parameter("gn_b", [C], f32, isOutput=False)
    dqkvw = nc.declare_dram_parameter("qkv_w", [3 * C, C], f32, isOutput=False)
    dqkvb = nc.declare_dram_parameter("qkv_b", [3 * C], f32, isOutput=False)
    dpw = nc.declare_dram_parameter("proj_w", [C, C], f32, isOutput=False)
    dpb = nc.declare_dram_parameter("proj_b", [C], f32, isOutput=False)
    dout = nc.declare_dram_parameter("out", [C, SC], f32, isOutput=True)
    if debug:
        dbg = {}
        for nm, shp, dt_ in [("dbg_pb2", [128, 2], f32), ("dbg_fT", [128, NTB], f32),
                             ("dbg_vt", [128, 512], f16),
                             ("dbg_exp", [128, 512], f16), ("dbg_usb", [128, 512], f32),
                             ("dbg_zsb", [128, 512], f32), ("dbg_zrh", [128, 512], f16),
                             ("dbg_stage", [128, 512], f16), ("dbg_ao", [128, SC], f16),
                             ("dbg_q", [128, 512], f16), ("dbg_k", [128, 512], f16)]:
            dbg[nm] = nc.declare_dram_parameter(nm, shp, dt_, isOutput=True)

    from contextlib import ExitStack
    with tile.TileContext(nc) as tc, ExitStack() as ctx:
        singles = ctx.enter_context(tc.tile_pool(name="singles", bufs=1))
        # PSUM pools: 2x3 + 1 + 1 = 8 banks exactly.
        ps_pool = ctx.enter_context(tc.tile_pool(name="ps", bufs=2, space="PSUM"))
        u_pool = ctx.enter_context(tc.tile_pool(name="u", bufs=1, space="PSUM"))
        zr_pool = ctx.enter_context(tc.tile_pool(name="zr", bufs=1, space="PSUM"))
        kq = ctx.enter_context(tc.tile_pool(name="kq", bufs=2))
        vt_pool = ctx.enter_context(tc.tile_pool(name="vtp", bufs=1))
        # preamble-scoped pools (released before expS allocates)
        pre = ExitStack()
        xp = pre.enter_context(tc.tile_pool(name="xp", bufs=2))
        hp = pre.enter_context(tc.tile_pool(name="hp", bufs=2))
        wstage = pre.enter_context(tc.tile_pool(name="wstage", bufs=2))

        ones = singles.tile([128, 512], f32, tag="ones", name="ones")
        nc.vector.memset(ones, 1.0)
        zero_b = singles.tile([128, 1], f32, tag="zb", name="zb")
        nc.vector.memset(zero_b, 0.0)
        expb = singles.tile([128, 1], f32, tag="expb", name="expb")
        nc.vector.memset(expb, EXP_SHIFT)
        ones16 = singles.tile([128, 128], f16, tag="ones16", name="ones16")
        nc.vector.memset(ones16, 1.0)
        ident = singles.tile([128, 128], f32, tag="ident", name="ident")
        make_identity(nc, ident)

        # ---- small parameter loads ----
        gw = [singles.tile([128, 1], f32, tag=f"gw{i}", name=f"gw{i}") for i in range(2)]
        gb = [singles.tile([128, 1], f32, tag=f"gb{i}", name=f"gb{i}") for i in range(2)]
        pb = [singles.tile([128, 1], f32, tag=f"pb{i}", name=f"pb{i}") for i in range(2)]
        bqT = [singles.tile([128, 1], f32, tag=f"bqT{i}", name=f"bqT{i}") for i in range(2)]
        bvT = [singles.tile([128, 1], f32, tag=f"bvT{i}", name=f"bvT{i}") for i in range(2)]
        for ct in range(2):
            nc.sync.dma_start(out=gw[ct], in_=dgw[128 * ct:128 * (ct + 1)])
            nc.sync.dma_start(out=gb[ct], in_=dgb[128 * ct:128 * (ct + 1)])
            nc.sync.dma_start(out=pb[ct], in_=dpb[128 * ct:128 * (ct + 1)])
            nc.sync.dma_start(out=bqT[ct], in_=dqkvb[128 * ct:128 * (ct + 1)])
            nc.sync.dma_start(out=bvT[ct], in_=dqkvb[2 * C + 128 * ct:2 * C + 128 * (ct + 1)])

        # ---- weight transposes (PE) ----
        # wqkvT[ct][c, o] = qkv_w[o, 128*ct + c]; fp16 tiles [128, 768]
        wqkvT = [singles.tile([128, 3 * C], f16, tag=f"wqkvT{i}", name=f"wqkvT{i}") for i in range(2)]
        wpT = [singles.tile([128, C], f16, tag=f"wpT{i}", name=f"wpT{i}") for i in range(2)]
        for rt in range(6):  # qkv_w row-tiles [128, 256]
            wle = wstage.tile([128, C], f32, tag="wle", name="wle")
            nc.sync.dma_start(out=wle, in_=dqkvw[128 * rt:128 * (rt + 1), :])
            for ct in range(2):
                pt = ps_pool.tile([128, 1536], f32, tag="ps", name="ps")
                nc.tensor.transpose(pt[:, 0:128], wle[:, 128 * ct:128 * (ct + 1)], ident)
                nc.vector.tensor_copy(
                    out=wqkvT[ct][:, 128 * rt:128 * (rt + 1)], in_=pt[:, 0:128])
        for rt in range(2):
            wle = wstage.tile([128, C], f32, tag="wle", name="wle")
            nc.sync.dma_start(out=wle, in_=dpw[128 * rt:128 * (rt + 1), :])
            for ct in range(2):
                pt = ps_pool.tile([128, 1536], f32, tag="ps", name="ps")
                nc.tensor.transpose(pt[:, 0:128], wle[:, 128 * ct:128 * (ct + 1)], ident)
                nc.vector.tensor_copy(
                    out=wpT[ct][:, 128 * rt:128 * (rt + 1)], in_=pt[:, 0:128])

        # pb2 = proj_b + proj_w @ bv  (exact fold of the v bias)
        bqT16 = [singles.tile([128, 1], f16, tag=f"bqT16_{i}", name=f"bqT16_{i}") for i in range(2)]
        bvT16 = [singles.tile([128, 1], f16, tag=f"bvT16_{i}", name=f"bvT16_{i}") for i in range(2)]
        for ct in range(2):
            nc.vector.tensor_copy(out=bqT16[ct], in_=bqT[ct])
            nc.vector.tensor_copy(out=bvT16[ct], in_=bvT[ct])
        pb2 = [singles.tile([128, 1], f32, tag=f"pb2_{i}", name=f"pb2_{i}") for i in range(2)]
        for ct in range(2):
            bvp = zr_pool.tile([128, 512], f32, tag="zr", name="bvp")
            for kc in range(2):
                nc.tensor.matmul(bvp[:, 0:1], wpT[kc][:, 128 * ct:128 * (ct + 1)],
                                 bvT16[kc], start=(kc == 0), stop=(kc == 1))
            nc.vector.tensor_add(out=pb2[ct], in0=pb[ct], in1=bvp[:, 0:1])
        if debug:
            pbs = wstage.tile([128, 2], f32, tag="pbs", name="pbs")
            nc.vector.tensor_copy(out=pbs[:, 0:1], in_=pb2[0])
            nc.vector.tensor_copy(out=pbs[:, 1:2], in_=pb2[1])
            nc.sync.dma_start(out=dbg["dbg_pb2"][:, :], in_=pbs)

        # ---- GroupNorm stats ----
        x_sb = [xp.tile([128, S], f32, tag="x", name="x") for _ in range(2)]
        h_sb = [hp.tile([128, S], f16, tag="h", name="h") for _ in range(2)]
        stats = [wstage.tile([128, 2], f32, tag=f"st{i}", name=f"st{i}") for i in range(2)]
        for ct in range(2):
            nc.sync.dma_start(out=x_sb[ct], in_=dx[128 * ct:128 * (ct + 1), :])
            nc.vector.tensor_reduce(
                out=stats[ct][:, 0:1], in_=x_sb[ct], axis=AX.X, op=ALU.add)
            # sum(x^2) in one ACT pass (h_sb is scratch, overwritten later)
            nc.scalar.activation(out=h_sb[ct], in_=x_sb[ct], func=AF.Square,
                                 accum_out=stats[ct][:, 1:2])
        st_ps = zr_pool.tile([1, 512], f32, tag="zr", name="zr")
        for ct in range(2):
            nc.tensor.transpose(st_ps[0:1, 128 * ct:128 * (ct + 1)],
                                stats[ct][:, 0:1], ident)
            nc.tensor.transpose(st_ps[0:1, 256 + 128 * ct:256 + 128 * (ct + 1)],
                                stats[ct][:, 1:2], ident)
        gstats = singles.tile([1, 16], f32, tag="gstats", name="gstats")
        nc.vector.tensor_reduce(
            out=gstats,
            in_=st_ps.rearrange("p (k g c) -> p k g c", k=2, g=G),
            axis=AX.X, op=ALU.add)
        mu = singles.tile([1, G], f32, tag="mu", name="mu")
        varv = singles.tile([1, G], f32, tag="varv", name="varv")
        rstd = singles.tile([1, G], f32, tag="rstd", name="rstd")
        mrs = singles.tile([1, G], f32, tag="mrs", name="mrs")
        eps_sb2 = singles.tile([128, 1], f32, tag="eps2", name="eps2")
        nc.vector.memset(eps_sb2, EPS)
        nc.vector.tensor_scalar_mul(out=mu, in0=gstats[:, 0:G], scalar1=1.0 / GN_N)
        nc.vector.tensor_scalar_mul(out=varv, in0=gstats[:, G:2 * G], scalar1=1.0 / GN_N)
        musq = singles.tile([1, G], f32, tag="musq", name="musq")
        nc.vector.tensor_mul(out=musq, in0=mu, in1=mu)
        nc.vector.tensor_sub(out=varv, in0=varv, in1=musq)
        vb_ps = zr_pool.tile([128, 512], f32, tag="zr", name="vb_ps")
        nc.tensor.matmul(vb_ps[:, 0:G], ones[0:1, 0:128], varv,
                         start=True, stop=True)
        sdb = singles.tile([128, G], f32, tag="sdb", name="sdb")
        nc.scalar.activation(out=sdb, in_=vb_ps[:, 0:G], func=AF.Sqrt,
                             bias=eps_sb2)
        nc.vector.reciprocal(out=rstd, in_=sdb[0:1, :])
        nc.vector.tensor_mul(out=mrs, in0=mu, in1=rstd)

        # broadcast rstd/mrs to per-channel A, Bb
        A = [singles.tile([128, 1], f32, tag=f"A{i}", name=f"A{i}") for i in range(2)]
        Bb = [singles.tile([128, 1], f32, tag=f"B{i}", name=f"B{i}") for i in range(2)]
        for ct in range(2):
            arep = zr_pool.tile([128, 2], f32, tag="zr", name="zr")
            for g4 in range(4):
                g = 4 * ct + g4
                nc.tensor.matmul(
                    arep[32 * g4:32 * (g4 + 1), 0:1], ones[0:1, 0:32],
                    rstd[:, g:g + 1], start=True, stop=True,
                    tile_position=(0, 32 * g4))
                nc.tensor.matmul(
                    arep[32 * g4:32 * (g4 + 1), 1:2], ones[0:1, 0:32],
                    mrs[:, g:g + 1], start=True, stop=True,
                    tile_position=(0, 32 * g4))
            nc.vector.tensor_mul(out=A[ct], in0=arep[:, 0:1], in1=gw[ct])
            tmp = wstage.tile([128, 1], f32, tag="tmpB", name="tmpB")
            nc.vector.tensor_mul(out=tmp, in0=arep[:, 1:2], in1=gw[ct])
            nc.vector.tensor_sub(out=Bb[ct], in0=gb[ct], in1=tmp)

        # ---- apply GN ----
        xq_sb = [xp.tile([128, SC], f32, tag="xq", name="xq") for _ in range(2)]
        hq_sb = [hp.tile([128, SC], f16, tag="hq", name="hq") for _ in range(2)]
        xpb = [singles.tile([128, SC], f32, tag=f"xpb{i}", name=f"xpb{i}") for i in range(2)]
        for ct in range(2):
            nc.sync.dma_start(out=xq_sb[ct], in_=dxq[128 * ct:128 * (ct + 1), :])
            nc.scalar.activation(out=h_sb[ct], in_=x_sb[ct], func=AF.Identity,
                                 bias=Bb[ct], scale=A[ct])
            nc.scalar.activation(out=hq_sb[ct], in_=xq_sb[ct], func=AF.Identity,
                                 bias=Bb[ct], scale=A[ct])
            nc.scalar.activation(out=xpb[ct], in_=xq_sb[ct], func=AF.Identity,
                                 bias=pb2[ct])

        # ---- q, k matmuls (fp16, no bias) ----
        q_sb = [kq.tile([128, SC], f16, tag="q", name="q") for _ in range(2)]
        k_sb = [kq.tile([128, S], f16, tag="k", name="k") for _ in range(2)]
        for ct in range(2):
            pq = ps_pool.tile([128, 1536], f32, tag="ps", name="ps")
            for n in range(2):
                for kc in range(2):
                    nc.tensor.matmul(
                        pq[:, 512 * n:512 * (n + 1)],
                        wqkvT[kc][:, 128 * ct:128 * (ct + 1)],
                        hq_sb[kc][:, 512 * n:512 * (n + 1)],
                        start=(kc == 0), stop=(kc == 1))
            nc.vector.tensor_scalar(out=q_sb[ct], in0=pq[:, 0:SC],
                                    scalar1=PRESCALE, scalar2=None, op0=ALU.mult)
        for ct in range(2):
            for chunk in range(4):  # 4 chunks of 1024 cols
                pk = ps_pool.tile([128, 1536], f32, tag="ps", name="ps")
                for n in range(2):
                    cl = 1024 * chunk + 512 * n
                    for kc in range(2):
                        nc.tensor.matmul(
                            pk[:, 512 * n:512 * (n + 1)],
                            wqkvT[kc][:, C + 128 * ct:C + 128 * (ct + 1)],
                            h_sb[kc][:, cl:cl + 512],
                            start=(kc == 0), stop=(kc == 1))
                nc.vector.tensor_copy(
                    out=k_sb[ct][:, 1024 * chunk:1024 * (chunk + 1)],
                    in_=pk[:, 0:1024])

        # ---- f_t = exp(SCALE * (k^T bq)_t): q-bias fold, t in partitions ----
        fps = zr_pool.tile([128, 512], f32, tag="zr", name="fps")
        for tb in range(NTB):
            for kc in range(2):
                nc.tensor.matmul(
                    fps[:, tb:tb + 1],
                    k_sb[kc][:, 128 * tb:128 * (tb + 1)], bqT16[kc],
                    start=(kc == 0), stop=(kc == 1))
        fT = singles.tile([128, NTB], f32, tag="fT", name="fT")
        nc.scalar.activation(out=fT, in_=fps[:, 0:NTB], func=AF.Exp,
                             scale=SCALE, bias=zero_b)
        if debug:
            nc.sync.dma_start(out=dbg["dbg_fT"][:, :], in_=fT)

        # ---- vt: [t(128p), tb, h, 64] fp16; col 0:32 = f*v, col 32 = f ----
        vt = vt_pool.tile([128, NTB, NH, 64], f16, tag="vt", name="vt")
        for tb in range(NTB):
            pv = (u_pool.tile([128, 512], f32, tag="u", name="pv")
                  if tb % 2 == 0 else
                  zr_pool.tile([128, 512], f32, tag="zr", name="pv"))
            for kc in range(2):
                nc.tensor.matmul(
                    pv[:, 0:256],
                    h_sb[kc][:, 128 * tb:128 * (tb + 1)],
                    wqkvT[kc][:, 2 * C:3 * C],
                    start=(kc == 0), stop=(kc == 1))
            nc.scalar.activation(
                out=vt[:, tb, :, 0:32],
                in_=pv[:, 0:256].rearrange("p (h d) -> p h d", h=NH),
                func=AF.Identity, bias=zero_b, scale=fT[:, tb:tb + 1])
        for h in range(NH):
            nc.gpsimd.tensor_copy(out=vt[:, :, h, 32:33].rearrange("p t o -> p (t o)"),
                                  in_=fT)
        if debug:
            nc.sync.dma_start(out=dbg["dbg_vt"][:, :],
                              in_=vt[:, 0, :, :].rearrange("p h d -> p (h d)"))
            nc.sync.dma_start(out=dbg["dbg_q"][:, :], in_=q_sb[0][:, 0:512])
            nc.sync.dma_start(out=dbg["dbg_k"][:, :], in_=k_sb[0][:, 0:512])

        # ---- attention ----
        pre.close()  # release x/h/staging SBUF for expS
        exps_pool = ctx.enter_context(tc.tile_pool(name="exps", bufs=1))
        rep = ctx.enter_context(tc.tile_pool(name="rep", bufs=2))
        zt_pool = ctx.enter_context(tc.tile_pool(name="zt", bufs=2))
        stg_pool = ctx.enter_context(tc.tile_pool(name="stg", bufs=2))
        scr_pool = ctx.enter_context(tc.tile_pool(name="scr", bufs=1))
        osb_pool = ctx.enter_context(tc.tile_pool(name="osb", bufs=2))
        expS = exps_pool.tile([128, 2, NTB, 512], f16, tag="expS", name="expS")
        m2t = exps_pool.tile([128, 1], f32, tag="m2t", name="m2t")
        nc.vector.memset(m2t, M2F)
        attnout = [kq.tile([128, SC], f16, tag="ao", name="ao") for _ in range(2)]
        # t-block groups of 3 (last group 2) per s-half
        groups = [list(range(i, min(i + 3, NTB))) for i in range(0, NTB, 3)]
        # Per-half DVE exp groups. Spacing between consecutive DVE chains
        # must exceed the ~4.4us chain latency (3+ slots at the 1.45us ACT
        # cadence) or the DVE queue backs up and queued custom ops hold
        # their PSUM score banks, stalling PE.
        DVE_GROUPS = {0: (3, 8), 1: (3, 8)}

        def emit_u(h, half, grp, first, last):
            for i, tb in enumerate(grp):
                nc.tensor.matmul(
                    U[64 * half:64 * half + 33, :],
                    vt[:, tb, h, 0:33],
                    expS[:, half, tb, :],
                    start=(first and i == 0),
                    stop=(last and i == len(grp) - 1),
                    tile_position=(0, 64 * half))

        reps = {}

        def prefetch_rep(hh):
            pkt, pkr = hh // 4, 32 * (hh % 4)
            k3p = rep.tile([96, S], f16, tag="k3", name="k3")
            q3p = rep.tile([96, SC], f16, tag="q3", name="q3")
            for i in range(3):
                nc.gpsimd.dma_start(out=k3p[32 * i:32 * (i + 1), :],
                                    in_=k_sb[pkt][pkr:pkr + 32, :])
                nc.gpsimd.dma_start(out=q3p[32 * i:32 * (i + 1), :],
                                    in_=q_sb[pkt][pkr:pkr + 32, :])
            reps[hh] = (k3p, q3p)

        prefetch_rep(0)
        pending_z = None
        for h in range(NH):
            kt, kr = h // 4, 32 * (h % 4)
            k3, q3 = reps.pop(h)
            U = u_pool.tile([128, 512], f32, tag="u", name="u")
            # Single 22-slot stream over (half, group): the U-drain FIFO lags
            # behind the exp stream, so half 0's U tail drains while half 1's
            # scores/exps flow and ACT never starves at the half boundary.
            nemit = {0: 0, 1: 0}
            fifo = []          # (half, grp) ready for U emission, in order
            pend_dve = []      # (ready_slot, half, grp)
            units = [(hf, gi) for hf in range(2) for gi in range(len(groups))]

            def drain(half_grp, last):
                half, g = half_grp
                emit_u(h, half, g, nemit[half] == 0, last)
                nemit[half] += len(g)

            for slot, (half, gi) in enumerate(units):
                grp = groups[gi]
                ng = len(grp)
                ps = ps_pool.tile([128, 1536], f32, tag="ps", name="ps")
                for i, tb in enumerate(grp):
                    nc.tensor.matmul(
                        ps[:, 512 * i:512 * (i + 1)],
                        k3[32 * i:32 * (i + 1), 128 * tb:128 * (tb + 1)],
                        q3[32 * i:32 * (i + 1), 512 * half:512 * (half + 1)],
                        start=True, stop=True, tile_position=(32 * i, 0))
                while pend_dve and pend_dve[0][0] <= slot:
                    _, hf, g = pend_dve.pop(0)
                    fifo.append((hf, g))
                if fifo:
                    drain(fifo.pop(0), False)
                if half == 0 and gi == 0 and pending_z is not None:
                    pending_z()
                    pending_z = None
                if half == 1 and gi == 0 and h + 1 < NH:
                    prefetch_rep(h + 1)
                if gi in DVE_GROUPS[half]:
                    # 4-pass fp16 exp2 bit-trick on DVE; pass 1 frees ps
                    scr1 = scr_pool.tile([128, 1536], f32, tag="scr1", name="scr1")
                    scr2 = scr_pool.tile([128, 1536], f32, tag="scr2", name="scr2")
                    nc.vector._custom_dve(
                        EXP_CORE, out=scr1[:, 0:512 * ng],
                        in0=ps[:, 0:512 * ng], in1=m2t,
                        s0=EXPQ2, s1=EXPQ1, imm2=512.0 + PS_SHIFT)
                    nc.vector.tensor_scalar(
                        out=scr2[:, 0:512 * ng], in0=scr1[:, 0:512 * ng],
                        scalar1=EXPK2 - PS_SHIFT, scalar2=None, op0=ALU.add)
                    nc.vector.tensor_scalar(
                        out=scr1.bitcast(i32)[:, 0:512 * ng],
                        in0=scr2.bitcast(i32)[:, 0:512 * ng],
                        scalar1=0x7FFFFF, scalar2=None,
                        op0=ALU.bitwise_and)
                    nc.vector.tensor_copy(
                        out=expS.bitcast(i16)[:, half, grp[0]:grp[0] + ng, :],
                        in_=scr1.bitcast(i32)[:, 0:512 * ng].rearrange(
                            "p (t s) -> p t s", s=512))
                    pend_dve.append((slot + 4, half, grp))
                else:
                    nc.scalar.activation(
                        out=expS[:, half, grp[0]:grp[0] + ng, :],
                        in_=ps[:, 0:512 * ng].rearrange("p (t s) -> p t s", s=512),
                        func=AF.Exp, scale=ACT_EXP_SCALE, bias=expb)
                    fifo.append((half, grp))
            tail = fifo + [(hf, g) for _, hf, g in pend_dve]
            left = {0: 0, 1: 0}
            for hf, g in tail:
                left[hf] += 1
            for hf, g in tail:
                left[hf] -= 1
                drain((hf, g), left[hf] == 0)
            # U -> SBUF; batched 1/Z on full tile (rows 32 & 96 are Z).
            # Only the U reader (u_sb copy) + DVE/Pool work emit now; the
            # PE broadcast matmuls are deferred past the next head's first
            # scores group so PE never waits on the recip chain.
            u_sb = zt_pool.tile([128, 512], f32, tag="usb", name="usb")
            nc.vector.tensor_copy(out=u_sb[0:33, :], in_=U[0:33, :])
            nc.vector.tensor_copy(out=u_sb[64:97, :], in_=U[64:97, :])
            zrc = zt_pool.tile([128, 512], f32, tag="zrc", name="zrc")
            zrh = zt_pool.tile([128, 512], f16, tag="zrh", name="zrh")
            nc.vector.reciprocal_approx_fast(out=zrc, in_=u_sb)
            nc.gpsimd.tensor_copy(out=zrh, in_=zrc)

            def z_tail(h=h, kt=kt, kr=kr, u_sb=u_sb, zrc=zrc, zrh=zrh):
                zrep = zr_pool.tile([128, 512], f32, tag="zr", name="zrep")
                nc.tensor.matmul(zrep[0:64, :], ones16[32:33, 0:64],
                                 zrh[32:33, :], start=True, stop=True,
                                 tile_position=(32, 0))
                nc.tensor.matmul(zrep[64:128, :], ones16[96:97, 64:128],
                                 zrh[96:97, :], start=True, stop=True,
                                 tile_position=(96, 64))
                stage = stg_pool.tile([128, 512], f16, tag="stage", name="stage")
                nc.vector.tensor_mul(out=stage, in0=u_sb, in1=zrep)
                if debug and h == 0:
                    nc.sync.dma_start(out=dbg["dbg_usb"][:, :], in_=u_sb)
                    nc.sync.dma_start(out=dbg["dbg_zsb"][:, :], in_=zrc)
                    nc.sync.dma_start(out=dbg["dbg_zrh"][:, :], in_=zrh)
                    nc.sync.dma_start(out=dbg["dbg_stage"][:, :], in_=stage)
                nc.gpsimd.dma_start(out=attnout[kt][kr:kr + 32, 0:512],
                                    in_=stage[0:32, :])
                nc.gpsimd.dma_start(out=attnout[kt][kr:kr + 32, 512:1024],
                                    in_=stage[64:96, :])

            pending_z = z_tail
        if pending_z is not None:
            pending_z()
            pending_z = None

        if debug:
            nc.sync.dma_start(out=dbg["dbg_ao"][:, :], in_=attnout[0])

        # ---- proj + residual ----
        out_sb = [osb_pool.tile([128, SC], f32, tag="osb", name="osb") for _ in range(2)]
        for ct in range(2):
            pp = ps_pool.tile([128, 1536], f32, tag="ps", name="ps")
            for n in range(2):
                for kc in range(2):
                    nc.tensor.matmul(
                        pp[:, 512 * n:512 * (n + 1)],
                        wpT[kc][:, 128 * ct:128 * (ct + 1)],
                        attnout[kc][:, 512 * n:512 * (n + 1)],
                        start=(kc == 0), stop=(kc == 1))
            nc.vector.tensor_add(out=out_sb[ct], in0=pp[:, 0:SC], in1=xpb[ct])
            nc.sync.dma_start(out=dout[128 * ct:128 * (ct + 1), :],
                              in_=out_sb[ct])

    nc.finalize()
    return nc


def make_in_maps(inputs):
    xf = np.ascontiguousarray(
        np.asarray(inputs["x"], dtype=np.float32)).reshape(B, C, S)
    in_maps = []
    for i in range(NCORES):
        b, sc = i // 4, SC * (i % 4)
        in_maps.append({
            "x_full": xf[b],
            "xq": np.ascontiguousarray(xf[b][:, sc:sc + SC]),
            "gn_w": np.asarray(inputs["gn_w"], np.float32),
            "gn_b": np.asarray(inputs["gn_b"], np.float32),
            "qkv_w": np.asarray(inputs["qkv_w"], np.float32),
            "qkv_b": np.asarray(inputs["qkv_b"], np.float32),
            "proj_w": np.asarray(inputs["proj_w"], np.float32),
            "proj_b": np.asarray(inputs["proj_b"], np.float32),
        })
    return in_maps


def kernel(x, gn_w, gn_b, qkv_w, qkv_b, proj_w, proj_b):
    from concourse.bass_utils import run_bass_kernel_spmd

    if "nc" not in _cache:
        _cache["nc"] = _build_nc()
    nc = _cache["nc"]

    in_maps = make_in_maps(dict(x=x, gn_w=gn_w, gn_b=gn_b, qkv_w=qkv_w,
                                qkv_b=qkv_b, proj_w=proj_w, proj_b=proj_b))
    res = run_bass_kernel_spmd(nc, in_maps, list(range(NCORES))).results
    out = np.empty((B, C, S), np.float32)
    for i in range(NCORES):
        b, sc = i // 4, SC * (i % 4)
        out[b][:, sc:sc + SC] = res[i]["out"]
    return out.reshape(B, C, 16, 16, 16)
